# revision 16
# baseline (speedup 1.0000x reference)
"""GroupRouter MoE routing kernel for 8 Trainium2 NeuronCores.

Problem: B=262144 tokens, D=512 features, G=4 groups x GS=4 experts, top-2.
  group_logits = x @ group_w.T + group_b            [B, 4]
  top_group    = argmax(group_logits)               [B]
  in_logits    = x @ in_w[top_group].T + in_b[..]   [B, 4]
  probs        = softmax(in_logits)                 [B, 4]
  (weights, in_idx) = top2(probs); expert = experts_table[top_group, in_idx]

Strategy: data-parallel over 8 cores (32768 tokens each).  The shard is
pre-transposed on the host to [D, BC] so features arrive on SBUF partitions
and the GEMM needs no on-chip transposes: each 128-token block of the x tile
is the matmul *stationary* operand ([128d, 128tok]) and the fused router
weight matrix Wc=[group_w; in_w] ([128d, 20]) is the *moving* operand, so the
[128tok, 20] logit block lands token-major in PSUM.  Logits accumulate in
SBUF per group of tiles and the argmax/softmax/top-2 postprocess runs as
segmented vector ops per group, overlapped with the DMA stream of later
tiles; tail tiles/groups shrink so little work remains after the last DMA.
Outputs (idx as f32 + weights, fused [.,4]) are PE-transposed per group into
a (k, kb, j) layout so output DMAs use contiguous 2KB descriptors.
kernel-token u = 16384*kb + 128*k + j; the host inverts that permutation
(cheap reshape) after gathering results.
"""

import numpy as np

import concourse.bacc as bacc
import concourse.tile as tile
import concourse.mybir as mybir
from concourse.bass_utils import run_bass_kernel_spmd

B, D, G, GS = 262144, 512, 4, 4
NO = G + G * GS            # 20 logit rows (4 group + 16 in-group)
NCORES = 8
BC = B // NCORES           # 32768 tokens per core
P = 128                    # SBUF partitions
CH = D // P                # 4 contraction chunks of 128
S = BC // P                # 256 slots of 128 tokens
KB = S // P                # 2 output half-blocks of 128 slots

# x-tile sizes (tokens).  Tail tiles shrink so the last matmul burst and
# postprocess after the final DMA are short.
TILES = [2048] * 15 + [1024, 512, 256, 256]
# postprocess groups as lists of tile indices (slots contiguous, and each
# group must stay within one 128-slot output half-block)
GROUPS = [[0, 1, 2, 3], [4, 5, 6, 7], [8, 9, 10, 11],
          [12], [13], [14], [15], [16], [17], [18]]
TAIL_DIRECT = (6, 7, 8, 9)  # group indices shipped without the PE transpose
assert sum(TILES) == BC

F32 = mybir.dt.float32
I32 = mybir.dt.int32
AX = mybir.AxisListType
OP = mybir.AluOpType

_cached_nc = {}


def _postprocess(nc, post, Lg, iout, wout, kconst, k4const, s0, sg):
    """Argmax/softmax/top-2 for slots [s0, s0+sg) given logits Lg [P, sg, NO].
    Writes iout/wout [:, s0:s0+sg, :] (both f32)."""
    Gv = Lg[:, :, 0:G]
    INv = Lg[:, :, G:NO].rearrange("p s (g k) -> p s g k", g=G)

    def bcast(t):  # [P, sg] -> [P, sg, 4] (stride-0 inner)
        return t[:, :].unsqueeze(2).broadcast_to([P, sg, 4])

    gmax = post.tile([P, sg], F32, tag="gmax", name="gmax")
    nc.vector.tensor_reduce(gmax, Gv, axis=AX.X, op=OP.max)
    eqg = post.tile([P, sg, G], F32, tag="eqg", name="eqg")
    nc.vector.tensor_tensor(eqg, Gv, bcast(gmax), op=OP.is_equal)
    # select chosen group's 4 in-logits: sum_g eq[g] * in[g, k]
    tmp = post.tile([P, sg, GS, G], F32, tag="tmp", name="tmp")
    nc.vector.tensor_tensor(
        tmp.rearrange("p s k g -> p s g k"),
        eqg.unsqueeze(3).broadcast_to([P, sg, G, GS]), INv, op=OP.mult)
    sel = post.tile([P, sg, GS], F32, tag="sel", name="sel")
    nc.vector.tensor_reduce(sel, tmp, axis=AX.X, op=OP.add)
    # softmax over the 4 selected logits
    e = post.tile([P, sg, GS], F32, tag="e", name="e")
    nc.scalar.activation(e, sel, func=mybir.ActivationFunctionType.Exp)
    ssum = post.tile([P, sg], F32, tag="ssum", name="ssum")
    nc.vector.tensor_reduce(ssum, e, axis=AX.X, op=OP.add)
    rcp = post.tile([P, sg], F32, tag="rcp", name="rcp")
    nc.vector.reciprocal(rcp, ssum)
    pr = post.tile([P, sg, GS], F32, tag="pr", name="pr")
    nc.vector.tensor_tensor(pr, e, bcast(rcp), op=OP.mult)
    # top-2 values + in-group indices
    p1 = wout[:, s0:s0 + sg, 0]
    nc.vector.tensor_reduce(p1, pr, axis=AX.X, op=OP.max)
    eq1 = post.tile([P, sg, GS], F32, tag="eq1", name="eq1")
    nc.vector.tensor_tensor(eq1, pr, bcast(p1), op=OP.is_equal)
    tk = post.tile([P, sg, GS], F32, tag="tk", name="tk")
    kb_ = kconst.unsqueeze(1).broadcast_to([P, sg, GS])
    nc.vector.tensor_tensor(tk, eq1, kb_, op=OP.mult)
    i1 = post.tile([P, sg], F32, tag="i1", name="i1")
    nc.vector.tensor_reduce(i1, tk, axis=AX.X, op=OP.add)
    pm = post.tile([P, sg, GS], F32, tag="pm", name="pm")
    nc.vector.scalar_tensor_tensor(pm, eq1, -1e30, pr, op0=OP.mult, op1=OP.add)
    p2 = wout[:, s0:s0 + sg, 1]
    nc.vector.tensor_reduce(p2, pm, axis=AX.X, op=OP.max)
    eq2 = post.tile([P, sg, GS], F32, tag="eq2", name="eq2")
    nc.vector.tensor_tensor(eq2, pm, bcast(p2), op=OP.is_equal)
    tk2 = post.tile([P, sg, GS], F32, tag="tk2", name="tk2")
    nc.vector.tensor_tensor(tk2, eq2, kb_, op=OP.mult)
    i2 = post.tile([P, sg], F32, tag="i2", name="i2")
    nc.vector.tensor_reduce(i2, tk2, axis=AX.X, op=OP.add)
    # group base index (4*g) from the group-argmax mask
    tg = post.tile([P, sg, G], F32, tag="tg", name="tg")
    nc.vector.tensor_tensor(tg, eqg,
                            k4const.unsqueeze(1).broadcast_to([P, sg, G]),
                            op=OP.mult)
    g4 = post.tile([P, sg], F32, tag="g4", name="g4")
    nc.vector.tensor_reduce(g4, tg, axis=AX.X, op=OP.add)
    nc.vector.tensor_tensor(iout[:, s0:s0 + sg, 0], g4, i1, op=OP.add)
    nc.vector.tensor_tensor(iout[:, s0:s0 + sg, 1], g4, i2, op=OP.add)


def _build(has_bias):
    nc = bacc.Bacc("TRN2", target_bir_lowering=False, num_devices=NCORES)
    x = nc.dram_tensor("x", [D, BC], F32, kind="ExternalInput")
    wt = nc.dram_tensor("wt", [D, NO], F32, kind="ExternalInput")
    if has_bias:
        bias = nc.dram_tensor("bias", [NO], F32, kind="ExternalInput")
    o_o = nc.dram_tensor("o_o", [BC, 4], F32, kind="ExternalOutput")
    # tail groups ship untransposed into a small token-ordered tensor
    n_tail = sum(TILES[i] for gi in TAIL_DIRECT for i in GROUPS[gi])
    o_tail = nc.dram_tensor("o_tail", [max(n_tail, 1), 4], F32,
                            kind="ExternalOutput")

    x_r = x.ap().rearrange("(c p) t -> p c t", p=P)          # [128, 4, 32768]
    # DRAM order (k, kb, j, q); host unpermutes token u = 16384*kb + 128*k + j
    o_r = o_o.ap().rearrange("(k kb j) q -> k kb j q", kb=KB, j=P)
    ot_r = o_tail.ap().rearrange("(s j) q -> j s q", j=P)    # [128, nt/128, 4]

    with tile.TileContext(nc) as tc:
        with (
            tc.tile_pool(name="singles", bufs=1) as singles,
            tc.tile_pool(name="xs_pool", bufs=3) as xs_pool,
            tc.tile_pool(name="lg_pool", bufs=2) as lg_pool,
            tc.tile_pool(name="post", bufs=2) as post,
            tc.tile_pool(name="outs", bufs=1) as outs,
            tc.tile_pool(name="ofin_pool", bufs=2) as ofin_pool,
            tc.tile_pool(name="ps_pool", bufs=2, space="PSUM") as ps_pool,
            tc.tile_pool(name="pt_pool", bufs=2, space="PSUM") as pt_pool,
        ):
            # ---- constants ----
            ident = singles.tile([P, P], F32)
            nc.vector.memset(ident, 1.0)
            nc.gpsimd.affine_select(
                ident, ident, pattern=[[-1, P]], base=0, channel_multiplier=1,
                compare_op=OP.is_equal, fill=0.0)
            # weight load on the gpsimd queue so x-tile 0's HWDGE setup (SP)
            # is not serialized behind it
            wt_sb = singles.tile([P, CH, NO], F32)
            nc.gpsimd.dma_start(out=wt_sb, in_=wt.ap().rearrange("(c p) j -> p c j", p=P))
            if has_bias:
                bias_sb = singles.tile([1, NO], F32)
                nc.sync.dma_start(out=bias_sb, in_=bias.ap().unsqueeze(0))
                ones = singles.tile([1, P], F32)
                nc.vector.memset(ones, 1.0)
            kconst = singles.tile([P, GS], F32)
            k4const = singles.tile([P, G], F32)
            for j in range(GS):
                nc.vector.memset(kconst[:, j:j + 1], float(j))
                nc.vector.memset(k4const[:, j:j + 1], float(j * GS))

            # fused per-token outputs: [idx0, idx1, w0, w1]
            out_sb = outs.tile([P, S, 4], F32)
            iout = out_sb[:, :, 0:2]
            wout = out_sb[:, :, 2:4]

            def ship_transposed(s0, sg):
                # PE-transpose [j, k] -> [k, j] so the output DMA uses
                # contiguous 2KB descriptors; DMA on the idle gpsimd queue so
                # its data wait never stalls the x-tile DMA stream
                kb = s0 // P
                k0 = s0 - kb * P
                pt = pt_pool.tile([P, 4, P], F32, tag="pt", name="pt")
                for q in range(4):
                    nc.tensor.transpose(
                        pt[0:sg, q, :], out_sb[:, s0:s0 + sg, q], ident)
                ofin = ofin_pool.tile([P, P, 4], F32, tag="ofin", name="ofin")
                nc.scalar.copy(ofin[0:sg, :, :],
                               pt[0:sg, :, :].rearrange("k q j -> k j q"))
                nc.gpsimd.dma_start(out=o_r[k0:k0 + sg, kb, :, :],
                                    in_=ofin[0:sg, :, :])

            tile_t0 = np.cumsum([0] + TILES)  # token offset per tile
            tail_s0 = (BC - n_tail) // P      # first slot shipped direct
            pending_ship = None               # deferred (s0, sg) ship
            for gi, grp in enumerate(GROUPS):
                sg = sum(TILES[i] for i in grp) // P
                s0 = int(tile_t0[grp[0]]) // P
                Lg = lg_pool.tile([P, sg, NO], F32, tag="Lg", name="Lg")
                loff = 0
                for i in grp:
                    tt = TILES[i]
                    blk = tt // P
                    t0 = int(tile_t0[i])
                    xs = xs_pool.tile([P, CH, tt], F32, tag="xs", name="xs")
                    nc.sync.dma_start(out=xs, in_=x_r[:, :, t0:t0 + tt])
                    ps = ps_pool.tile([P, blk, 32], F32, tag="ps", name="ps")
                    for kk in range(blk):
                        for c in range(CH):
                            nc.tensor.matmul(
                                ps[:, kk, 0:NO], xs[:, c, P * kk:P * (kk + 1)],
                                wt_sb[:, c, :], start=(c == 0),
                                stop=(c == CH - 1 and not has_bias))
                        if has_bias:
                            nc.tensor.matmul(ps[:, kk, 0:NO], ones, bias_sb,
                                             start=False, stop=True)
                    nc.scalar.copy(Lg[:, loff:loff + blk, :], ps[:, :, 0:NO])
                    loff += blk
                # emit the previous group's transpose-ship only after this
                # group's matmuls so its pending PE transposes (waiting on
                # the previous postprocess) never clog PE's wait queue ahead
                # of this group's matmuls
                if pending_ship is not None:
                    ship_transposed(*pending_ship)
                    pending_ship = None
                _postprocess(nc, post, Lg, iout, wout, kconst, k4const, s0, sg)
                if gi in TAIL_DIRECT:
                    # tail group: ship token-ordered with small descriptors,
                    # skipping the transpose+copy chain entirely
                    st = s0 - tail_s0
                    nc.sync.dma_start(out=ot_r[:, st:st + sg, :],
                                      in_=out_sb[:, s0:s0 + sg, :])
                else:
                    pending_ship = (s0, sg)
            if pending_ship is not None:
                ship_transposed(*pending_ship)
    nc.finalize()
    return nc


def _get_nc(has_bias):
    if has_bias not in _cached_nc:
        _cached_nc[has_bias] = _build(has_bias)
    return _cached_nc[has_bias]


def kernel(routing_features, group_w, group_b, in_w, in_b, experts_table,
           trace=False):
    x = np.asarray(routing_features, np.float32)
    gw = np.asarray(group_w, np.float32)
    gb = np.asarray(group_b, np.float32)
    iw = np.asarray(in_w, np.float32).reshape(G * GS, D)
    ib = np.asarray(in_b, np.float32).reshape(G * GS)
    table = np.asarray(experts_table, np.int32).reshape(-1)

    wt = np.ascontiguousarray(np.concatenate([gw, iw], 0).T)   # [512, 20]
    bias = np.concatenate([gb, ib], 0)                          # [20]
    has_bias = bool(np.any(bias))

    # shard along batch, then transpose each shard so features sit on the
    # DRAM rows the kernel maps to SBUF partitions
    xt = np.ascontiguousarray(x.reshape(NCORES, BC, D).transpose(0, 2, 1))

    in_maps = []
    for c in range(NCORES):
        m = {"x": xt[c], "wt": wt}
        if has_bias:
            m["bias"] = bias
        in_maps.append(m)
    nc = _get_nc(has_bias)
    try:
        res = run_bass_kernel_spmd(nc, in_maps,
                                   core_ids=list(range(NCORES)), trace=trace)
    except (ImportError, ModuleNotFoundError):
        # NTFF profiling hook unavailable in this environment; run untraced.
        res = run_bass_kernel_spmd(nc, in_maps,
                                   core_ids=list(range(NCORES)), trace=False)
    n_tail = sum(TILES[i] for gi in TAIL_DIRECT for i in GROUPS[gi])
    idx_parts = []
    w_parts = []
    for c in range(NCORES):
        # DRAM order (k, kb, j, q) -> token u = 16384*kb + 128*k + j
        oc = res.results[c]["o_o"].reshape(P, KB, P, 4)
        oc = oc.transpose(1, 0, 2, 3).reshape(BC, 4).copy()
        if n_tail:
            oc[BC - n_tail:] = res.results[c]["o_tail"]
        idx_parts.append(oc[:, 0:2].astype(np.int32))
        w_parts.append(oc[:, 2:4])
    idx = np.concatenate(idx_parts, 0)
    w = np.ascontiguousarray(np.concatenate(w_parts, 0))
    expert_indices = table[idx]                                 # int32 [B, 2]
    if trace:
        kernel.last_exec_time_ns = res.exec_time_ns
        if kernel.last_exec_time_ns is None:
            # No hardware NTFF in this environment: fall back to the
            # CoreSim cost-model timeline (single core, SPMD-identical).
            try:
                from concourse.timeline_sim import TimelineSim
                kernel.last_exec_time_ns = int(TimelineSim(nc).simulate())
                kernel.time_source = "cost-model timeline sim"
            except Exception:
                pass
        else:
            kernel.time_source = "ntff"
    return expert_indices, w


# revision 18
# speedup vs baseline: 1.0190x; 1.0190x over previous
"""GroupRouter MoE routing kernel for 8 Trainium2 NeuronCores.

Problem: B=262144 tokens, D=512 features, G=4 groups x GS=4 experts, top-2.
  group_logits = x @ group_w.T + group_b            [B, 4]
  top_group    = argmax(group_logits)               [B]
  in_logits    = x @ in_w[top_group].T + in_b[..]   [B, 4]
  probs        = softmax(in_logits)                 [B, 4]
  (weights, in_idx) = top2(probs); expert = experts_table[top_group, in_idx]

Strategy: data-parallel over 8 cores (32768 tokens each).  The shard is
pre-transposed on the host to [D, BC] so features arrive on SBUF partitions
and the GEMM needs no on-chip transposes: each 128-token block of the x tile
is the matmul *stationary* operand ([128d, 128tok]) and the fused router
weight matrix Wc=[group_w; in_w] ([128d, 20]) is the *moving* operand, so the
[128tok, 20] logit block lands token-major in PSUM.  Logits accumulate in
SBUF per group of tiles and the argmax/softmax/top-2 postprocess runs as
segmented vector ops per group, overlapped with the DMA stream of later
tiles; tail tiles/groups shrink so little work remains after the last DMA.
Outputs (idx as f32 + weights, fused [.,4]) are PE-transposed per group into
a (k, kb, j) layout so output DMAs use contiguous 2KB descriptors.
kernel-token u = 16384*kb + 128*k + j; the host inverts that permutation
(cheap reshape) after gathering results.
"""

import numpy as np

import concourse.bacc as bacc
import concourse.tile as tile
import concourse.mybir as mybir
from concourse.bass_utils import run_bass_kernel_spmd

B, D, G, GS = 262144, 512, 4, 4
NO = G + G * GS            # 20 logit rows (4 group + 16 in-group)
NCORES = 8
BC = B // NCORES           # 32768 tokens per core
P = 128                    # SBUF partitions
CH = D // P                # 4 contraction chunks of 128
S = BC // P                # 256 slots of 128 tokens
KB = S // P                # 2 output half-blocks of 128 slots

# x-tile sizes (tokens).  Tail tiles shrink so the last matmul burst and
# postprocess after the final DMA are short.
TILES = [2048] * 15 + [1024, 512, 256, 256]
# postprocess groups as lists of tile indices (slots contiguous, and each
# group must stay within one 128-slot output half-block)
GROUPS = [[0, 1, 2, 3], [4, 5, 6, 7], [8, 9, 10, 11],
          [12], [13], [14], [15], [16], [17], [18]]
TAIL_DIRECT = (6, 7, 8, 9)  # group indices shipped without the PE transpose
# transpose-shipped slot blocks (s0, sg, emit_after_group): emitted after
# that group's matmuls, by which point the feeding postprocess has long
# finished, so the pending PE transposes never clog PE's 4-deep wait queue
SHIP_BLOCKS = [(0, 64, 1), (64, 64, 2), (128, 64, 3), (192, 48, 6)]
assert sum(TILES) == BC

F32 = mybir.dt.float32
I32 = mybir.dt.int32
AX = mybir.AxisListType
OP = mybir.AluOpType

_cached_nc = {}


def _postprocess(nc, post, Lg, iout, wout, kconst, k4const, s0, sg):
    """Argmax/softmax/top-2 for slots [s0, s0+sg) given logits Lg [P, sg, NO].
    Writes iout/wout [:, s0:s0+sg, :] (both f32)."""
    Gv = Lg[:, :, 0:G]
    INv = Lg[:, :, G:NO].rearrange("p s (g k) -> p s g k", g=G)

    def bcast(t):  # [P, sg] -> [P, sg, 4] (stride-0 inner)
        return t[:, :].unsqueeze(2).broadcast_to([P, sg, 4])

    gmax = post.tile([P, sg], F32, tag="gmax", name="gmax")
    nc.vector.tensor_reduce(gmax, Gv, axis=AX.X, op=OP.max)
    eqg = post.tile([P, sg, G], F32, tag="eqg", name="eqg")
    nc.vector.tensor_tensor(eqg, Gv, bcast(gmax), op=OP.is_equal)
    # select chosen group's 4 in-logits: sum_g eq[g] * in[g, k]
    tmp = post.tile([P, sg, GS, G], F32, tag="tmp", name="tmp")
    nc.vector.tensor_tensor(
        tmp.rearrange("p s k g -> p s g k"),
        eqg.unsqueeze(3).broadcast_to([P, sg, G, GS]), INv, op=OP.mult)
    sel = post.tile([P, sg, GS], F32, tag="sel", name="sel")
    nc.vector.tensor_reduce(sel, tmp, axis=AX.X, op=OP.add)
    # softmax over the 4 selected logits
    e = post.tile([P, sg, GS], F32, tag="e", name="e")
    nc.scalar.activation(e, sel, func=mybir.ActivationFunctionType.Exp)
    ssum = post.tile([P, sg], F32, tag="ssum", name="ssum")
    nc.vector.tensor_reduce(ssum, e, axis=AX.X, op=OP.add)
    rcp = post.tile([P, sg], F32, tag="rcp", name="rcp")
    nc.vector.reciprocal(rcp, ssum)
    pr = post.tile([P, sg, GS], F32, tag="pr", name="pr")
    nc.vector.tensor_tensor(pr, e, bcast(rcp), op=OP.mult)
    # top-2 values + in-group indices
    p1 = wout[:, s0:s0 + sg, 0]
    nc.vector.tensor_reduce(p1, pr, axis=AX.X, op=OP.max)
    eq1 = post.tile([P, sg, GS], F32, tag="eq1", name="eq1")
    nc.vector.tensor_tensor(eq1, pr, bcast(p1), op=OP.is_equal)
    tk = post.tile([P, sg, GS], F32, tag="tk", name="tk")
    kb_ = kconst.unsqueeze(1).broadcast_to([P, sg, GS])
    nc.vector.tensor_tensor(tk, eq1, kb_, op=OP.mult)
    i1 = post.tile([P, sg], F32, tag="i1", name="i1")
    nc.vector.tensor_reduce(i1, tk, axis=AX.X, op=OP.add)
    pm = post.tile([P, sg, GS], F32, tag="pm", name="pm")
    nc.vector.scalar_tensor_tensor(pm, eq1, -1e30, pr, op0=OP.mult, op1=OP.add)
    p2 = wout[:, s0:s0 + sg, 1]
    nc.vector.tensor_reduce(p2, pm, axis=AX.X, op=OP.max)
    eq2 = post.tile([P, sg, GS], F32, tag="eq2", name="eq2")
    nc.vector.tensor_tensor(eq2, pm, bcast(p2), op=OP.is_equal)
    tk2 = post.tile([P, sg, GS], F32, tag="tk2", name="tk2")
    nc.vector.tensor_tensor(tk2, eq2, kb_, op=OP.mult)
    i2 = post.tile([P, sg], F32, tag="i2", name="i2")
    nc.vector.tensor_reduce(i2, tk2, axis=AX.X, op=OP.add)
    # group base index (4*g) from the group-argmax mask
    tg = post.tile([P, sg, G], F32, tag="tg", name="tg")
    nc.vector.tensor_tensor(tg, eqg,
                            k4const.unsqueeze(1).broadcast_to([P, sg, G]),
                            op=OP.mult)
    g4 = post.tile([P, sg], F32, tag="g4", name="g4")
    nc.vector.tensor_reduce(g4, tg, axis=AX.X, op=OP.add)
    nc.vector.tensor_tensor(iout[:, s0:s0 + sg, 0], g4, i1, op=OP.add)
    nc.vector.tensor_tensor(iout[:, s0:s0 + sg, 1], g4, i2, op=OP.add)


def _build(has_bias):
    nc = bacc.Bacc("TRN2", target_bir_lowering=False, num_devices=NCORES)
    x = nc.dram_tensor("x", [D, BC], F32, kind="ExternalInput")
    wt = nc.dram_tensor("wt", [D, NO], F32, kind="ExternalInput")
    if has_bias:
        bias = nc.dram_tensor("bias", [NO], F32, kind="ExternalInput")
    o_o = nc.dram_tensor("o_o", [BC, 4], F32, kind="ExternalOutput")
    # tail groups ship untransposed into a small token-ordered tensor
    n_tail = sum(TILES[i] for gi in TAIL_DIRECT for i in GROUPS[gi])
    o_tail = nc.dram_tensor("o_tail", [max(n_tail, 1), 4], F32,
                            kind="ExternalOutput")

    x_r = x.ap().rearrange("(c p) t -> p c t", p=P)          # [128, 4, 32768]
    # DRAM order (k, kb, j, q); host unpermutes token u = 16384*kb + 128*k + j
    o_r = o_o.ap().rearrange("(k kb j) q -> k kb j q", kb=KB, j=P)
    ot_r = o_tail.ap().rearrange("(s j) q -> j s q", j=P)    # [128, nt/128, 4]

    with tile.TileContext(nc) as tc:
        with (
            tc.tile_pool(name="singles", bufs=1) as singles,
            tc.tile_pool(name="xs_pool", bufs=3) as xs_pool,
            tc.tile_pool(name="lg_pool", bufs=2) as lg_pool,
            tc.tile_pool(name="post", bufs=2) as post,
            tc.tile_pool(name="outs", bufs=1) as outs,
            tc.tile_pool(name="ofin_pool", bufs=2) as ofin_pool,
            tc.tile_pool(name="ps_pool", bufs=2, space="PSUM") as ps_pool,
            tc.tile_pool(name="pt_pool", bufs=2, space="PSUM") as pt_pool,
        ):
            # ---- constants ----
            ident = singles.tile([P, P], F32)
            nc.vector.memset(ident, 1.0)
            nc.gpsimd.affine_select(
                ident, ident, pattern=[[-1, P]], base=0, channel_multiplier=1,
                compare_op=OP.is_equal, fill=0.0)
            # weight load on the gpsimd queue so x-tile 0's HWDGE setup (SP)
            # is not serialized behind it
            wt_sb = singles.tile([P, CH, NO], F32)
            nc.gpsimd.dma_start(out=wt_sb, in_=wt.ap().rearrange("(c p) j -> p c j", p=P))
            if has_bias:
                bias_sb = singles.tile([1, NO], F32)
                nc.sync.dma_start(out=bias_sb, in_=bias.ap().unsqueeze(0))
                ones = singles.tile([1, P], F32)
                nc.vector.memset(ones, 1.0)
            kconst = singles.tile([P, GS], F32)
            k4const = singles.tile([P, G], F32)
            for j in range(GS):
                nc.vector.memset(kconst[:, j:j + 1], float(j))
                nc.vector.memset(k4const[:, j:j + 1], float(j * GS))

            # fused per-token outputs: [idx0, idx1, w0, w1]
            out_sb = outs.tile([P, S, 4], F32)
            iout = out_sb[:, :, 0:2]
            wout = out_sb[:, :, 2:4]

            def ship_transposed(s0, sg):
                # PE-transpose [j, k] -> [k, j] so the output DMA uses
                # contiguous 2KB descriptors; DMA on the idle gpsimd queue so
                # its data wait never stalls the x-tile DMA stream
                kb = s0 // P
                k0 = s0 - kb * P
                pt = pt_pool.tile([P, 4, P], F32, tag="pt", name="pt")
                for q in range(4):
                    nc.tensor.transpose(
                        pt[0:sg, q, :], out_sb[:, s0:s0 + sg, q], ident)
                ofin = ofin_pool.tile([P, P, 4], F32, tag="ofin", name="ofin")
                nc.scalar.copy(ofin[0:sg, :, :],
                               pt[0:sg, :, :].rearrange("k q j -> k j q"))
                nc.gpsimd.dma_start(out=o_r[k0:k0 + sg, kb, :, :],
                                    in_=ofin[0:sg, :, :])

            tile_t0 = np.cumsum([0] + TILES)  # token offset per tile
            tail_s0 = (BC - n_tail) // P      # first slot shipped direct
            for gi, grp in enumerate(GROUPS):
                sg = sum(TILES[i] for i in grp) // P
                s0 = int(tile_t0[grp[0]]) // P
                Lg = lg_pool.tile([P, sg, NO], F32, tag="Lg", name="Lg")
                loff = 0
                for i in grp:
                    tt = TILES[i]
                    blk = tt // P
                    t0 = int(tile_t0[i])
                    xs = xs_pool.tile([P, CH, tt], F32, tag="xs", name="xs")
                    nc.sync.dma_start(out=xs, in_=x_r[:, :, t0:t0 + tt])
                    ps = ps_pool.tile([P, blk, 32], F32, tag="ps", name="ps")
                    for kk in range(blk):
                        for c in range(CH):
                            nc.tensor.matmul(
                                ps[:, kk, 0:NO], xs[:, c, P * kk:P * (kk + 1)],
                                wt_sb[:, c, :], start=(c == 0),
                                stop=(c == CH - 1 and not has_bias))
                        if has_bias:
                            nc.tensor.matmul(ps[:, kk, 0:NO], ones, bias_sb,
                                             start=False, stop=True)
                    nc.scalar.copy(Lg[:, loff:loff + blk, :], ps[:, :, 0:NO])
                    loff += blk
                for bs0, bsg, bemit in SHIP_BLOCKS:
                    if bemit == gi:
                        ship_transposed(bs0, bsg)
                _postprocess(nc, post, Lg, iout, wout, kconst, k4const, s0, sg)
                if gi in TAIL_DIRECT:
                    # tail group: ship token-ordered with small descriptors,
                    # skipping the transpose+copy chain entirely
                    st = s0 - tail_s0
                    nc.sync.dma_start(out=ot_r[:, st:st + sg, :],
                                      in_=out_sb[:, s0:s0 + sg, :])
    nc.finalize()
    return nc


def _get_nc(has_bias):
    if has_bias not in _cached_nc:
        _cached_nc[has_bias] = _build(has_bias)
    return _cached_nc[has_bias]


def kernel(routing_features, group_w, group_b, in_w, in_b, experts_table,
           trace=False):
    x = np.asarray(routing_features, np.float32)
    gw = np.asarray(group_w, np.float32)
    gb = np.asarray(group_b, np.float32)
    iw = np.asarray(in_w, np.float32).reshape(G * GS, D)
    ib = np.asarray(in_b, np.float32).reshape(G * GS)
    table = np.asarray(experts_table, np.int32).reshape(-1)

    wt = np.ascontiguousarray(np.concatenate([gw, iw], 0).T)   # [512, 20]
    bias = np.concatenate([gb, ib], 0)                          # [20]
    has_bias = bool(np.any(bias))

    # shard along batch, then transpose each shard so features sit on the
    # DRAM rows the kernel maps to SBUF partitions
    xt = np.ascontiguousarray(x.reshape(NCORES, BC, D).transpose(0, 2, 1))

    in_maps = []
    for c in range(NCORES):
        m = {"x": xt[c], "wt": wt}
        if has_bias:
            m["bias"] = bias
        in_maps.append(m)
    nc = _get_nc(has_bias)
    try:
        res = run_bass_kernel_spmd(nc, in_maps,
                                   core_ids=list(range(NCORES)), trace=trace)
    except (ImportError, ModuleNotFoundError):
        # NTFF profiling hook unavailable in this environment; run untraced.
        res = run_bass_kernel_spmd(nc, in_maps,
                                   core_ids=list(range(NCORES)), trace=False)
    n_tail = sum(TILES[i] for gi in TAIL_DIRECT for i in GROUPS[gi])
    idx_parts = []
    w_parts = []
    for c in range(NCORES):
        # DRAM order (k, kb, j, q) -> token u = 16384*kb + 128*k + j
        oc = res.results[c]["o_o"].reshape(P, KB, P, 4)
        oc = oc.transpose(1, 0, 2, 3).reshape(BC, 4).copy()
        if n_tail:
            oc[BC - n_tail:] = res.results[c]["o_tail"]
        idx_parts.append(oc[:, 0:2].astype(np.int32))
        w_parts.append(oc[:, 2:4])
    idx = np.concatenate(idx_parts, 0)
    w = np.ascontiguousarray(np.concatenate(w_parts, 0))
    expert_indices = table[idx]                                 # int32 [B, 2]
    if trace:
        kernel.last_exec_time_ns = res.exec_time_ns
        if kernel.last_exec_time_ns is None:
            # No hardware NTFF in this environment: fall back to the
            # CoreSim cost-model timeline (single core, SPMD-identical).
            try:
                from concourse.timeline_sim import TimelineSim
                kernel.last_exec_time_ns = int(TimelineSim(nc).simulate())
                kernel.time_source = "cost-model timeline sim"
            except Exception:
                pass
        else:
            kernel.time_source = "ntff"
    return expert_indices, w
